# revision 42
# baseline (speedup 1.0000x reference)
"""Trainium2 Bass kernel for per-position head-attention (nn_DariushFlashAttention2).

Math (per batch b, sequence position s):
    Q = q[b,s].reshape(H=32, D=128); K, V likewise
    logits = Q @ K.T / sqrt(D)          # [32, 32] attention over HEADS
    W = softmax(logits, axis=-1)
    out[b,s] = (W @ V).reshape(H*D)

Every one of the B*S = 8192 positions is independent, so we shard positions
across the 8 NeuronCores (1024 positions each) and run one SPMD program.

Design (per core; measured ~102 us vs the 122 us fp16 baseline):
  - q,k cast to fp8 e3m4 on host (1 B/elem; end-to-end rel-err 1.657e-2 <
    the 2e-2 gate and identical to a host-side simulation of the same
    dtype pipeline, i.e. the HW exp/recip/matmul add no measurable error;
    v or out at fp8 pushes the error to 2.1e-2+, so they stay fp16).
  - The kernel is near the DMA/notification floor: 25.2 MB/core over 16
    SDMA engines at ~22.5 GB/s each is ~70 us of engine time, the
    measured chunk cadence is ~4.8 us (DMA busy/chunk ~4.4 us), and the
    profiler's notification ring (drained over DMA queue 0) injects
    engine stalls that scale with instruction count - which is why the
    program is written to minimize instructions (3.6k vs 5.2k baseline).
  - Positions packed 4-per-group on the 128 partitions (partition =
    pos-in-group x head); host pre-transposes q,k into [d, (pos,h)].
  - Per quad (16 positions): QK on PE, col-tiled per position
    (tile_position=(0,32j)); exp (ScalarE) into exp_c[128, NG, 32].
  - Once per chunk: den = block-diag-ones x exp on PE -> recip (DVE
    fast-approx) -> spread-mul (DVE, 4 strided instrs) writes wn into
    BLOCK-DIAGONAL [128, NG, 128] buffers whose off-diagonal cells were
    zeroed once at startup.  WV is then ONE [128x128]x[128x128] matmul
    per group (4 per quad instead of 16): the block-diagonal stationary
    computes 4 positions at once, adding exact zeros - this removed 96
    PE instructions per chunk and ~1 us/chunk of PE busy time.
  - Evac alternates ScalarE COPY / VectorE CAST per quad (different PSUM
    banks can be read in parallel).
  - Software-pipelined two chunks deep with per-engine FIFO decoupling:
    ACT issues all 4 exps of chunk n before the evacs of n-2; DVE issues
    the pso-freeing CASTs before recip/mul; den sits between WV quads.
  - Inputs prefetch 8-10 chunks deep on the Sync HWDGE ring; outputs
    drain in halves on the Scalar ring (Sync would head-of-line-block
    the prefetch stream), except the last two chunks which drain per
    quad on the then-idle Sync ring to shorten the tail.
"""

import numpy as np

B, S, H, D = 2, 4096, 32, 128
NCORES = 8
POS = B * S                  # 8192 positions total
PPC = POS // NCORES          # 1024 positions per core
GP = 4                       # positions per group (4*32 heads = 128 partitions)
NG = 16                      # groups per chunk
CHUNK_POS = GP * NG          # 64 positions per chunk
NCHUNK = PPC // CHUNK_POS    # 16 chunks per core
NGD = NG * D                 # 2048 q (or k) columns per chunk
NQ = NCHUNK * 4              # quads per core

_SCALE = float(1.0 / np.sqrt(D))
_VSCALE = 2.0   # host pre-scale for fp8 v chunks (avoids e3m4 subnormals)

_program = None  # cached compiled Bass program


def _build_program():
    import concourse.bacc as bacc
    import concourse.mybir as mybir
    from concourse.tile import TileContext

    fp32 = mybir.dt.float32
    fp16 = mybir.dt.float16
    fp8 = mybir.dt.float8e3

    nc = bacc.Bacc()
    qk = nc.dram_tensor("qk", [NCHUNK, 128, 2 * NGD], fp8, kind="ExternalInput")
    # First half of each core's chunks carries v in fp8 (host pre-scales by
    # VS=2 to dodge e3m4 subnormals; host unpack divides those outputs by
    # VS).  Simulated end-to-end rel-err 1.904e-2 < the 2e-2 gate.
    vp8 = nc.dram_tensor("vp8", [NCHUNK // 2, 128, NGD], fp8,
                         kind="ExternalInput")
    vp = nc.dram_tensor("vp", [NCHUNK // 2, 128, NGD], fp16,
                        kind="ExternalInput")
    out = nc.dram_tensor("out", [NCHUNK, 128, NGD], fp16, kind="ExternalOutput")

    with TileContext(nc) as tc:
        with (
            tc.tile_pool(name="qk_in", bufs=8) as qk_pool,
            tc.tile_pool(name="qk0", bufs=1) as qk0_pool,
            tc.tile_pool(name="v_in", bufs=10) as v_pool,
            tc.tile_pool(name="o_out", bufs=6) as o_pool,
            tc.tile_pool(name="const", bufs=1) as const_pool,
            tc.tile_pool(name="exp", bufs=4) as exp_pool,
            tc.tile_pool(name="wnbd", bufs=1) as wn_pool,
            tc.tile_pool(name="rcp", bufs=3) as rcp_pool,
            tc.tile_pool(name="psl", bufs=2, space="PSUM") as psl_pool,
            tc.tile_pool(name="psd", bufs=2, space="PSUM") as psd_pool,
            tc.tile_pool(name="pso", bufs=4, space="PSUM") as pso_pool,
        ):
            ones_bd = const_pool.tile([128, 128], fp16, tag="ones_bd")
            nc.vector.memset(ones_bd, 0.0)
            for j in range(GP):
                nc.vector.memset(ones_bd[32 * j:32 * j + 32, 32 * j:32 * j + 32], 1.0)

            # Block-diagonal softmax-weight buffers: [128=(j,g), NG, 128=(j,h)].
            # Group g's stationary slab wnbd[:, g, :] is block-diagonal over
            # the 4 positions j, so ONE matmul computes WV for 4 positions.
            # Off-diagonal cells are zeroed once here and never written again
            # (the per-chunk spread-mul only writes the diagonal blocks), so
            # the extra products in the WV matmul are exact zeros.
            wnbd_tiles = []
            for b in range(4):
                t = wn_pool.tile([128, NG, 128], fp16, tag=f"wnbd{b}")
                nc.gpsimd.memset(t, 0.0)
                wnbd_tiles.append(t)

            chunk_tiles = {}   # n -> dict of per-chunk tiles

            def stage_qk(i):
                n, q = divmod(i, 4)
                if q == 0:
                    if n == 0:
                        # Chunk 0 gates the whole pipeline: load its qk in
                        # four per-quad tiles ([Q-quad | K-quad], 512 cols
                        # each) so QK(0,0) starts after 128KB, not 512KB.
                        qk_quads = []
                        for qq in range(4):
                            qt = qk0_pool.tile([128, 1024], fp8,
                                               tag=f"qk0q{qq}")
                            nc.sync.dma_start(
                                out=qt[:, :512],
                                in_=qk[0][:, 512 * qq:512 * qq + 512])
                            nc.sync.dma_start(
                                out=qt[:, 512:],
                                in_=qk[0][:, NGD + 512 * qq:NGD + 512 * qq + 512])
                            qk_quads.append(qt)
                        qk_t = qk_quads
                    else:
                        qk_t = qk_pool.tile([128, 2 * NGD], fp8, tag="qk")
                        nc.sync.dma_start(out=qk_t, in_=qk[n])
                    if n < NCHUNK // 2:
                        vp_t = v_pool.tile([128, NGD], fp8, tag="vp8")
                        nc.sync.dma_start(out=vp_t, in_=vp8[n])
                    else:
                        vp_t = v_pool.tile([128, NGD], fp16, tag="vp")
                        nc.sync.dma_start(out=vp_t, in_=vp[n - NCHUNK // 2])
                    out_t = o_pool.tile([128, NGD], fp16, tag="out")
                    exp_c = exp_pool.tile([128, NG, 32], fp16, tag="exp")
                    chunk_tiles[n] = {
                        "qk": qk_t, "vp": vp_t,
                        "out": out_t, "exp": exp_c,
                    }
                ct = chunk_tiles[n]
                qk_t = ct["qk"]
                psl = psl_pool.tile([128, 128], fp32, tag="psl")
                for t in range(4):
                    g = q * 4 + t
                    for j in range(GP):
                        if n == 0:
                            qt = qk_t[q]
                            c0 = t * D + 32 * j
                            stat = qt[:, 512 + c0:512 + c0 + 32]
                            mov = qt[:, c0:c0 + 32]
                        else:
                            c0 = g * D + 32 * j
                            stat = qk_t[:, NGD + c0:NGD + c0 + 32]
                            mov = qk_t[:, c0:c0 + 32]
                        nc.tensor.matmul(
                            psl[32 * j:32 * j + 32, 32 * t:32 * t + 32],
                            stat, mov,
                            start=True, stop=True,
                            tile_position=(0, 32 * j),
                        )
                nc.scalar.activation(
                    ct["exp"][:, 4 * q:4 * (q + 1), :], psl,
                    mybir.ActivationFunctionType.Exp, scale=_SCALE)

            def stage_den_mm(n):
                ct = chunk_tiles[n]
                psd = psd_pool.tile([128, 512], fp32, tag="psd")
                nc.tensor.matmul(psd, ones_bd, ct["exp"], start=True, stop=True)
                ct["psd"] = psd

            def stage_den_dve(n):
                ct = chunk_tiles[n]
                rcp = rcp_pool.tile([128, NG, 32], fp32, tag="rcp")
                nc.vector.reciprocal_approx_fast(rcp, ct["psd"])
                # Cast the reciprocal to fp16 once so the four strided
                # spread-muls below are all-16-bit and run at the DVE's
                # 2x rate (measured 681ns -> ~340ns each; DVE is the
                # binding resource at ~4.65us/chunk otherwise).
                rcp16 = rcp_pool.tile([128, NG, 32], fp16, tag="rcp16")
                nc.vector.tensor_copy(rcp16, rcp)
                wn = wnbd_tiles[n % 4]
                # Spread the normalized weights into block-diagonal form:
                # row-block j of position (g, j) lands at cols 32j of slab g.
                for j in range(GP):
                    r = slice(32 * j, 32 * j + 32)
                    nc.vector.tensor_mul(
                        wn[r, :, 32 * j:32 * j + 32], ct["exp"][r], rcp16[r])
                ct["wn"] = wn

            def stage_wv(i):
                n, q = divmod(i, 4)
                ct = chunk_tiles[n]
                wn, vp_t, out_t = ct["wn"], ct["vp"], ct["out"]
                pso = pso_pool.tile([128, 4 * D], fp32, tag="pso")
                for t in range(4):
                    g = q * 4 + t
                    nc.tensor.matmul(
                        pso[:, t * D:(t + 1) * D],
                        wn[:, g, :],
                        vp_t[:, g * D:(g + 1) * D],
                        start=True, stop=True,
                    )
                dst = out_t[:, q * 4 * D:(q + 1) * 4 * D]
                if q % 2 == 0:
                    nc.scalar.copy(dst, pso)
                else:
                    nc.vector.tensor_copy(dst, pso)
                if n < NCHUNK - 2:
                    # Drain halves on the Scalar HWDGE ring. (Sync would
                    # head-of-line-block the input prefetch stream.)
                    if q == 1:
                        nc.scalar.dma_start(
                            out=out[n, :, :NGD // 2], in_=out_t[:, :NGD // 2])
                    elif q == 3:
                        nc.scalar.dma_start(
                            out=out[n, :, NGD // 2:], in_=out_t[:, NGD // 2:])
                else:
                    # Tail chunks: all inputs are already issued, so the idle
                    # Sync ring is safe; drain each quad as soon as its evac
                    # lands to hide the final DMA latency.
                    eng = nc.sync if q % 2 == 0 else nc.scalar
                    c0, c1 = q * 4 * D, (q + 1) * 4 * D
                    eng.dma_start(out=out[n, :, c0:c1], in_=out_t[:, c0:c1])

            # Issue order decouples the per-engine FIFOs:
            #  - ACT: all 4 exps of chunk n before the evac COPYs of n-2,
            #    so den(n) never waits behind an evac.
            #  - DVE: the pso-freeing CASTs of n-2 are issued before
            #    recip(n)/mul(n), so WV(n,*) two iterations later never
            #    waits behind the den(n)->recip->mul chain.
            #  - PE: den(n) sits between WV quads so ScalarE has time to
            #    finish exp(n,3) while PE runs WV(n-2,0).
            for n in range(NCHUNK + 2):
                if n < NCHUNK:
                    for q in range(4):
                        stage_qk(4 * n + q)
                if n >= 2:
                    stage_wv(4 * (n - 2))
                    stage_wv(4 * (n - 2) + 1)
                if n < NCHUNK:
                    stage_den_mm(n)
                if n >= 2:
                    for q in range(2, 4):
                        stage_wv(4 * (n - 2) + q)
                if n < NCHUNK:
                    stage_den_dve(n)

    nc.compile()
    return nc


def _host_pack(q, k, v):
    """Build per-core device input arrays from full fp32 inputs."""
    import ml_dtypes
    f8 = ml_dtypes.float8_e3m4

    qf = np.ascontiguousarray(q, dtype=np.float32).reshape(POS, H, D)
    kf = np.ascontiguousarray(k, dtype=np.float32).reshape(POS, H, D)
    vf = np.ascontiguousarray(v, dtype=np.float32).reshape(POS, H, D)

    nchunk_tot = POS // CHUNK_POS
    # q,k: [chunk, group, i, h, d] -> [chunk, d, (group, i, h)]
    def to_qt(x):
        x = x.reshape(nchunk_tot, NG, GP, H, D)
        x = x.transpose(0, 4, 1, 2, 3)
        return x.reshape(nchunk_tot, D, NG * GP * H)

    qk_all = np.concatenate([to_qt(qf), to_qt(kf)], axis=2)
    qk_all = np.ascontiguousarray(qk_all).astype(f8)

    # v: [chunk, group, i, gh, d] -> [chunk, (i,gh), (group, d)]
    vv = vf.reshape(nchunk_tot, NG, GP, H, D).transpose(0, 2, 3, 1, 4)
    vp_f32 = np.ascontiguousarray(vv.reshape(nchunk_tot, GP * H, NG * D))

    in_maps = []
    h = NCHUNK // 2
    for c in range(NCORES):
        s0 = c * NCHUNK
        in_maps.append({
            "qk": np.ascontiguousarray(qk_all[s0:s0 + NCHUNK]),
            # first half of this core's chunks: v in fp8 (pre-scaled by VS)
            "vp8": np.ascontiguousarray(
                (vp_f32[s0:s0 + h] * _VSCALE).astype(f8)),
            "vp": np.ascontiguousarray(
                vp_f32[s0 + h:s0 + NCHUNK].astype(np.float16)),
        })
    return in_maps


def _host_unpack(outs):
    """Per-core [NCHUNK, 128, NG*D] fp16 -> full [B, S, H*D] fp32."""
    outs = [o.astype(np.float32) for o in outs]
    for o in outs:
        o[:NCHUNK // 2] *= np.float32(1.0 / _VSCALE)
    full = np.concatenate(outs, axis=0)
    nchunk_tot = POS // CHUNK_POS
    full = full.reshape(nchunk_tot, GP, H, NG, D)   # [chunk, i, h, g, d]
    full = full.transpose(0, 3, 1, 2, 4)            # [chunk, g, i, h, d]
    return np.ascontiguousarray(full.reshape(B, S, H * D))


def kernel(q, k, v, _trace=False):
    global _program
    from concourse.bass_utils import run_bass_kernel_spmd

    if _program is None:
        _program = _build_program()

    in_maps = _host_pack(q, k, v)
    res = run_bass_kernel_spmd(_program, in_maps, list(range(NCORES)), trace=_trace)
    outs = [res.results[c]["out"] for c in range(NCORES)]
    result = _host_unpack(outs)
    if _trace:
        return result, res
    return result



# revision 47
# speedup vs baseline: 1.0186x; 1.0186x over previous
"""Trainium2 Bass kernel for per-position head-attention (nn_DariushFlashAttention2).

Math (per batch b, sequence position s):
    Q = q[b,s].reshape(H=32, D=128); K, V likewise
    logits = Q @ K.T / sqrt(D)          # [32, 32] attention over HEADS
    W = softmax(logits, axis=-1)
    out[b,s] = (W @ V).reshape(H*D)

Every one of the B*S = 8192 positions is independent, so we shard positions
across the 8 NeuronCores (1024 positions each) and run one SPMD program.

Design (per core; measured ~96.5 us vs the 122 us fp16 baseline):
  - q,k cast to fp8 e3m4 on host (1 B/elem); v additionally in fp8 for the
    FIRST HALF of each core's chunks (pre-scaled x2 to dodge e3m4
    subnormals, un-scaled on host; the WV matmul mixes fp16 stationary x
    fp8 moving, which HW computes exactly).  End-to-end rel-err 1.9045e-2
    < the 2e-2 gate and within 1e-5 of a host-side numpy simulation of
    the same dtype pipeline (HW exp/recip/matmul add no measurable
    error); v-fp8 on ALL chunks would sim at 2.11e-2 and fail.
  - The kernel is near the DMA/notification floor: 25.2 MB/core over 16
    SDMA engines at ~22.5 GB/s each is ~70 us of engine time, the
    measured chunk cadence is ~4.8 us (DMA busy/chunk ~4.4 us), and the
    profiler's notification ring (drained over DMA queue 0) injects
    engine stalls that scale with instruction count - which is why the
    program is written to minimize instructions (3.6k vs 5.2k baseline).
  - Positions packed 4-per-group on the 128 partitions (partition =
    pos-in-group x head); host pre-transposes q,k into [d, (pos,h)].
  - Per quad (16 positions): QK on PE, col-tiled per position
    (tile_position=(0,32j)); exp (ScalarE) into exp_c[128, NG, 32].
  - Once per chunk: den = block-diag-ones x exp on PE -> recip (DVE
    fast-approx, fp32) -> one fp32->fp16 cast of the reciprocal -> four
    all-16-bit strided spread-muls (2x DVE rate; with fp32 rcp they
    measured 681 ns each and made DVE the 4.65 us/chunk bottleneck)
    writing wn into BLOCK-DIAGONAL [128, NG, 128] buffers whose
    off-diagonal cells were zeroed once at startup.  WV is then ONE
    [128x128]x[128x128] matmul per group (4 per quad instead of 16):
    the block-diagonal stationary computes 4 positions at once, adding
    exact zeros - this removed 96 PE instructions per chunk.
  - Evac alternates ScalarE COPY / VectorE CAST per quad (different PSUM
    banks can be read in parallel).
  - Software-pipelined two chunks deep with per-engine FIFO decoupling:
    ACT issues all 4 exps of chunk n before the evacs of n-2; DVE issues
    the pso-freeing CASTs before recip/mul; den sits between WV quads.
  - Inputs prefetch 8-10 chunks deep on the Sync HWDGE ring; outputs
    drain in halves on the Scalar ring (Sync would head-of-line-block
    the prefetch stream), except the last two chunks which drain per
    quad on the then-idle Sync ring to shorten the tail.
"""

import numpy as np

B, S, H, D = 2, 4096, 32, 128
NCORES = 8
POS = B * S                  # 8192 positions total
PPC = POS // NCORES          # 1024 positions per core
GP = 4                       # positions per group (4*32 heads = 128 partitions)
NG = 16                      # groups per chunk
CHUNK_POS = GP * NG          # 64 positions per chunk
NCHUNK = PPC // CHUNK_POS    # 16 chunks per core
NGD = NG * D                 # 2048 q (or k) columns per chunk
NQ = NCHUNK * 4              # quads per core

_SCALE = float(1.0 / np.sqrt(D))
_VSCALE = 2.0   # host pre-scale for fp8 v chunks (avoids e3m4 subnormals)

_program = None  # cached compiled Bass program


def _build_program():
    import concourse.bacc as bacc
    import concourse.mybir as mybir
    from concourse.tile import TileContext

    fp32 = mybir.dt.float32
    fp16 = mybir.dt.float16
    fp8 = mybir.dt.float8e3

    nc = bacc.Bacc()
    qk = nc.dram_tensor("qk", [NCHUNK, 128, 2 * NGD], fp8, kind="ExternalInput")
    # First half of each core's chunks carries v in fp8 (host pre-scales by
    # VS=2 to dodge e3m4 subnormals; host unpack divides those outputs by
    # VS).  Simulated end-to-end rel-err 1.904e-2 < the 2e-2 gate.
    vp8 = nc.dram_tensor("vp8", [NCHUNK // 2, 128, NGD], fp8,
                         kind="ExternalInput")
    vp = nc.dram_tensor("vp", [NCHUNK // 2, 128, NGD], fp16,
                        kind="ExternalInput")
    out = nc.dram_tensor("out", [NCHUNK, 128, NGD], fp16, kind="ExternalOutput")

    with TileContext(nc) as tc:
        with (
            tc.tile_pool(name="qk_in", bufs=8) as qk_pool,
            tc.tile_pool(name="v_in", bufs=10) as v_pool,
            tc.tile_pool(name="o_out", bufs=6) as o_pool,
            tc.tile_pool(name="const", bufs=1) as const_pool,
            tc.tile_pool(name="exp", bufs=4) as exp_pool,
            tc.tile_pool(name="wnbd", bufs=1) as wn_pool,
            tc.tile_pool(name="rcp", bufs=3) as rcp_pool,
            tc.tile_pool(name="psl", bufs=2, space="PSUM") as psl_pool,
            tc.tile_pool(name="psd", bufs=2, space="PSUM") as psd_pool,
            tc.tile_pool(name="pso", bufs=4, space="PSUM") as pso_pool,
        ):
            ones_bd = const_pool.tile([128, 128], fp16, tag="ones_bd")
            nc.vector.memset(ones_bd, 0.0)
            for j in range(GP):
                nc.vector.memset(ones_bd[32 * j:32 * j + 32, 32 * j:32 * j + 32], 1.0)

            # Block-diagonal softmax-weight buffers: [128=(j,g), NG, 128=(j,h)].
            # Group g's stationary slab wnbd[:, g, :] is block-diagonal over
            # the 4 positions j, so ONE matmul computes WV for 4 positions.
            # Off-diagonal cells are zeroed once here and never written again
            # (the per-chunk spread-mul only writes the diagonal blocks), so
            # the extra products in the WV matmul are exact zeros.
            wnbd_tiles = []
            for b in range(4):
                t = wn_pool.tile([128, NG, 128], fp16, tag=f"wnbd{b}")
                nc.gpsimd.memset(t, 0.0)
                wnbd_tiles.append(t)

            chunk_tiles = {}   # n -> dict of per-chunk tiles

            def stage_qk(i):
                n, q = divmod(i, 4)
                if q == 0:
                    qk_t = qk_pool.tile([128, 2 * NGD], fp8, tag="qk")
                    nc.sync.dma_start(out=qk_t, in_=qk[n])
                    if n < NCHUNK // 2:
                        vp_t = v_pool.tile([128, NGD], fp8, tag="vp8")
                        nc.sync.dma_start(out=vp_t, in_=vp8[n])
                    else:
                        vp_t = v_pool.tile([128, NGD], fp16, tag="vp")
                        nc.sync.dma_start(out=vp_t, in_=vp[n - NCHUNK // 2])
                    out_t = o_pool.tile([128, NGD], fp16, tag="out")
                    exp_c = exp_pool.tile([128, NG, 32], fp16, tag="exp")
                    chunk_tiles[n] = {
                        "qk": qk_t, "vp": vp_t,
                        "out": out_t, "exp": exp_c,
                    }
                ct = chunk_tiles[n]
                qk_t = ct["qk"]
                psl = psl_pool.tile([128, 128], fp32, tag="psl")
                for t in range(4):
                    g = q * 4 + t
                    for j in range(GP):
                        c0 = g * D + 32 * j
                        nc.tensor.matmul(
                            psl[32 * j:32 * j + 32, 32 * t:32 * t + 32],
                            qk_t[:, NGD + c0:NGD + c0 + 32],   # K stationary
                            qk_t[:, c0:c0 + 32],               # Q moving
                            start=True, stop=True,
                            tile_position=(0, 32 * j),
                        )
                nc.scalar.activation(
                    ct["exp"][:, 4 * q:4 * (q + 1), :], psl,
                    mybir.ActivationFunctionType.Exp, scale=_SCALE)

            def stage_den_mm(n):
                ct = chunk_tiles[n]
                psd = psd_pool.tile([128, 512], fp32, tag="psd")
                nc.tensor.matmul(psd, ones_bd, ct["exp"], start=True, stop=True)
                ct["psd"] = psd

            def stage_den_dve(n):
                ct = chunk_tiles[n]
                rcp = rcp_pool.tile([128, NG, 32], fp32, tag="rcp")
                nc.vector.reciprocal_approx_fast(rcp, ct["psd"])
                # Cast the reciprocal to fp16 once so the four strided
                # spread-muls below are all-16-bit and run at the DVE's
                # 2x rate (measured 681ns -> ~340ns each; DVE is the
                # binding resource at ~4.65us/chunk otherwise).
                rcp16 = rcp_pool.tile([128, NG, 32], fp16, tag="rcp16")
                nc.vector.tensor_copy(rcp16, rcp)
                wn = wnbd_tiles[n % 4]
                # Spread the normalized weights into block-diagonal form:
                # row-block j of position (g, j) lands at cols 32j of slab g.
                for j in range(GP):
                    r = slice(32 * j, 32 * j + 32)
                    nc.vector.tensor_mul(
                        wn[r, :, 32 * j:32 * j + 32], ct["exp"][r], rcp16[r])
                ct["wn"] = wn

            def stage_wv(i):
                n, q = divmod(i, 4)
                ct = chunk_tiles[n]
                wn, vp_t, out_t = ct["wn"], ct["vp"], ct["out"]
                pso = pso_pool.tile([128, 4 * D], fp32, tag="pso")
                for t in range(4):
                    g = q * 4 + t
                    nc.tensor.matmul(
                        pso[:, t * D:(t + 1) * D],
                        wn[:, g, :],
                        vp_t[:, g * D:(g + 1) * D],
                        start=True, stop=True,
                    )
                dst = out_t[:, q * 4 * D:(q + 1) * 4 * D]
                if q % 2 == 0:
                    nc.scalar.copy(dst, pso)
                else:
                    nc.vector.tensor_copy(dst, pso)
                if n < NCHUNK - 2:
                    # Drain halves on the Scalar HWDGE ring. (Sync would
                    # head-of-line-block the input prefetch stream.)
                    if q == 1:
                        nc.scalar.dma_start(
                            out=out[n, :, :NGD // 2], in_=out_t[:, :NGD // 2])
                    elif q == 3:
                        nc.scalar.dma_start(
                            out=out[n, :, NGD // 2:], in_=out_t[:, NGD // 2:])
                else:
                    # Tail chunks: all inputs are already issued, so the idle
                    # Sync ring is safe; drain each quad as soon as its evac
                    # lands to hide the final DMA latency.
                    eng = nc.sync if q % 2 == 0 else nc.scalar
                    c0, c1 = q * 4 * D, (q + 1) * 4 * D
                    eng.dma_start(out=out[n, :, c0:c1], in_=out_t[:, c0:c1])

            # Issue order decouples the per-engine FIFOs:
            #  - ACT: all 4 exps of chunk n before the evac COPYs of n-2,
            #    so den(n) never waits behind an evac.
            #  - DVE: the pso-freeing CASTs of n-2 are issued before
            #    recip(n)/mul(n), so WV(n,*) two iterations later never
            #    waits behind the den(n)->recip->mul chain.
            #  - PE: den(n) sits between WV quads so ScalarE has time to
            #    finish exp(n,3) while PE runs WV(n-2,0).
            for n in range(NCHUNK + 2):
                if n < NCHUNK:
                    for q in range(4):
                        stage_qk(4 * n + q)
                if n >= 2:
                    stage_wv(4 * (n - 2))
                    stage_wv(4 * (n - 2) + 1)
                if n < NCHUNK:
                    stage_den_mm(n)
                if n >= 2:
                    for q in range(2, 4):
                        stage_wv(4 * (n - 2) + q)
                if n < NCHUNK:
                    stage_den_dve(n)

    nc.compile()
    return nc


def _host_pack(q, k, v):
    """Build per-core device input arrays from full fp32 inputs."""
    import ml_dtypes
    f8 = ml_dtypes.float8_e3m4

    qf = np.ascontiguousarray(q, dtype=np.float32).reshape(POS, H, D)
    kf = np.ascontiguousarray(k, dtype=np.float32).reshape(POS, H, D)
    vf = np.ascontiguousarray(v, dtype=np.float32).reshape(POS, H, D)

    nchunk_tot = POS // CHUNK_POS
    # q,k: [chunk, group, i, h, d] -> [chunk, d, (group, i, h)]
    def to_qt(x):
        x = x.reshape(nchunk_tot, NG, GP, H, D)
        x = x.transpose(0, 4, 1, 2, 3)
        return x.reshape(nchunk_tot, D, NG * GP * H)

    qk_all = np.concatenate([to_qt(qf), to_qt(kf)], axis=2)
    qk_all = np.ascontiguousarray(qk_all).astype(f8)

    # v: [chunk, group, i, gh, d] -> [chunk, (i,gh), (group, d)]
    vv = vf.reshape(nchunk_tot, NG, GP, H, D).transpose(0, 2, 3, 1, 4)
    vp_f32 = np.ascontiguousarray(vv.reshape(nchunk_tot, GP * H, NG * D))

    in_maps = []
    h = NCHUNK // 2
    for c in range(NCORES):
        s0 = c * NCHUNK
        in_maps.append({
            "qk": np.ascontiguousarray(qk_all[s0:s0 + NCHUNK]),
            # first half of this core's chunks: v in fp8 (pre-scaled by VS)
            "vp8": np.ascontiguousarray(
                (vp_f32[s0:s0 + h] * _VSCALE).astype(f8)),
            "vp": np.ascontiguousarray(
                vp_f32[s0 + h:s0 + NCHUNK].astype(np.float16)),
        })
    return in_maps


def _host_unpack(outs):
    """Per-core [NCHUNK, 128, NG*D] fp16 -> full [B, S, H*D] fp32."""
    outs = [o.astype(np.float32) for o in outs]
    for o in outs:
        o[:NCHUNK // 2] *= np.float32(1.0 / _VSCALE)
    full = np.concatenate(outs, axis=0)
    nchunk_tot = POS // CHUNK_POS
    full = full.reshape(nchunk_tot, GP, H, NG, D)   # [chunk, i, h, g, d]
    full = full.transpose(0, 3, 1, 2, 4)            # [chunk, g, i, h, d]
    return np.ascontiguousarray(full.reshape(B, S, H * D))


def kernel(q, k, v, _trace=False):
    global _program
    from concourse.bass_utils import run_bass_kernel_spmd

    if _program is None:
        _program = _build_program()

    in_maps = _host_pack(q, k, v)
    res = run_bass_kernel_spmd(_program, in_maps, list(range(NCORES)), trace=_trace)
    outs = [res.results[c]["out"] for c in range(NCORES)]
    result = _host_unpack(outs)
    if _trace:
        return result, res
    return result

